# revision 6
# baseline (speedup 1.0000x reference)
"""Embedding lookup (nn_LookupNetwork) on 8 Trainium2 NeuronCores.

Strategy: data-parallel over the batch; each core handles 512 of the 4096
batch rows (102,400 lookups) with a replicated table in local HBM.

The gather uses the Q7 `dma_gather` SWDGE instruction (one descriptor per
looked-up row, streamed through all 16 SDMA engines) instead of per-column
indirect DMAs. dma_gather indices are int16, so the 100k-row table is
re-laid-out on the host into 4 chunks of 32767 rows + 1 zero row each
(device table [4*32768, 128]); every position is gathered once per chunk,
with out-of-chunk positions (and the -1 sentinel) pointed at the chunk's
zero row. The 4 gathered tiles are summed on the DVE - no masking needed.

dma_gather writes element i to partition i%128, block i//128; the host
pre-permutes the index slots so that the summed SBUF tile is already in
natural row order per partition, making the output store a contiguous
128 x 16KB HWDGE DMA and the host-side unshard a pure reshape.
"""

import sys

sys.path.insert(0, "/opt/trn_rl_repo")

from contextlib import ExitStack

import numpy as np

import concourse.bacc as bacc
import concourse.bass as bass
import concourse.mybir as mybir
import concourse.tile as tile
from concourse.bass_utils import run_bass_kernel_spmd

VOCAB, D = 100000, 128
BATCH, HIST = 4096, 200
NCORES = 8
P = 128

V_CH = 32767            # table rows per chunk (int16 max + zero row)
NCH = 4                 # chunks needed to cover VOCAB
TBL_CH = 32768          # device-table rows per chunk (incl. zero row)
S = 4096                # positions gathered per tile
BLK = S // P            # output rows per partition per tile
NPOS = BATCH * HIST // NCORES   # 102400 positions per core
NT = NPOS // S          # tiles per core

_nc_cache = {}


def build_nc(nt=NT, bufs=2, reps=1):
    """reps>1 repeats the whole workload in-program (for timing: the
    per-exec HW time is (t(reps=R) - t(reps=1)) / (R - 1), which cancels
    the host/axon dispatch overhead)."""
    nc = bacc.Bacc(
        "TRN2", target_bir_lowering=False, debug=False, enable_asserts=False
    )
    idx_d = nc.dram_tensor(
        "idx", [P, nt * NCH * (S // 16)], mybir.dt.int16, kind="ExternalInput"
    ).ap()
    tab_d = nc.dram_tensor(
        "table", [NCH * TBL_CH, D], mybir.dt.float32, kind="ExternalInput"
    ).ap()
    out_d = nc.dram_tensor(
        "out", [nt, P, S], mybir.dt.float32, kind="ExternalOutput"
    ).ap()

    cols_t = NCH * (S // 16)        # idx columns per tile (all 4 chunks)
    with tile.TileContext(nc) as tc:
        with ExitStack() as ctx:
            ipool = ctx.enter_context(tc.tile_pool(name="ipool", bufs=3))
            gpools = [
                ctx.enter_context(tc.tile_pool(name=f"g{c}", bufs=bufs))
                for c in range(NCH)
            ]
            for t in [t for _ in range(reps) for t in range(nt)]:
                idx_t = ipool.tile([P, cols_t], mybir.dt.int16)
                nc.scalar.dma_start(
                    idx_t[:], idx_d[:, t * cols_t : (t + 1) * cols_t]
                )
                gs = []
                for c in range(NCH):
                    g = gpools[c].tile([P, S], mybir.dt.float32)
                    g3 = g[:].rearrange("p (b d) -> p b d", d=D)
                    nc.gpsimd.dma_gather(
                        g3,
                        tab_d[c * TBL_CH : (c + 1) * TBL_CH, :],
                        idx_t[:, c * (S // 16) : (c + 1) * (S // 16)],
                        S,
                        S,
                        D,
                        # >64 descriptors per lane per instruction cannot be
                        # coalesced into one SDMA packet
                        single_packet=False,
                    )
                    gs.append(g)
                nc.vector.tensor_tensor(
                    out=gs[0][:], in0=gs[0][:], in1=gs[1][:],
                    op=mybir.AluOpType.add,
                )
                nc.vector.tensor_tensor(
                    out=gs[2][:], in0=gs[2][:], in1=gs[3][:],
                    op=mybir.AluOpType.add,
                )
                nc.vector.tensor_tensor(
                    out=gs[0][:], in0=gs[0][:], in1=gs[2][:],
                    op=mybir.AluOpType.add,
                )
                nc.sync.dma_start(out_d[t], gs[0][:])
    nc.compile()
    return nc


def _get_nc():
    if "nc" not in _nc_cache:
        _nc_cache["nc"] = build_nc()
    return _nc_cache["nc"]


def _prep_table(table):
    t4 = np.zeros((NCH * TBL_CH, D), np.float32)
    for c in range(NCH):
        lo = c * V_CH
        hi = min(lo + V_CH, VOCAB)
        t4[c * TBL_CH : c * TBL_CH + (hi - lo)] = table[lo:hi]
    return t4


def _in_maps(input_batch, table, nt=NT):
    # slot i (within a tile) holds position p*BLK+b with p=i%128, b=i//128,
    # so dma_gather's i%128-partition placement lands rows in natural order
    # per partition. wrap-16: slot i sits at idx column i//16, partition i%16.
    v = np.asarray(input_batch).astype(np.int64).reshape(NCORES, NT, P, BLK)
    v = v[:, :nt]
    slots = v.transpose(0, 1, 3, 2).reshape(NCORES, nt, S)
    w = slots.reshape(NCORES, nt, S // 16, 16).transpose(0, 1, 3, 2)
    chunks = [
        np.where((w >= 0) & (w // V_CH == c), w - c * V_CH, V_CH).astype(np.int16)
        for c in range(NCH)
    ]
    idx = np.stack(chunks, axis=2)              # [core, nt, NCH, 16, 256]
    # device layout: partition q (= q%16 replica), column t*NCH*256 + c*256 + s
    idx = idx.transpose(0, 3, 1, 2, 4)          # [core, 16, nt, NCH, 256]
    idx = idx.reshape(NCORES, 16, nt * NCH * (S // 16))
    idx = np.ascontiguousarray(np.tile(idx, (1, 8, 1)))   # replicate to 128
    t4 = _prep_table(np.asarray(table, dtype=np.float32))
    return [{"idx": idx[c], "table": t4} for c in range(NCORES)]


def kernel(input_batch, table):
    nc = _get_nc()
    in_maps = _in_maps(input_batch, table)
    res = run_bass_kernel_spmd(nc, in_maps, list(range(NCORES)))
    return np.concatenate(
        [
            res.results[c]["out"].reshape(NPOS // HIST, HIST, D)
            for c in range(NCORES)
        ],
        axis=0,
    )


def run_traced(input_batch, table, trace_cores=None, tmpdir=None):
    """Run once with NTFF profiling; returns (output, BassKernelResults)."""
    nc = _get_nc()
    in_maps = _in_maps(input_batch, table)
    res = run_bass_kernel_spmd(
        nc, in_maps, list(range(NCORES)), trace=True,
        trace_cores=trace_cores, tmpdir=tmpdir,
    )
    out = np.concatenate(
        [
            res.results[c]["out"].reshape(NPOS // HIST, HIST, D)
            for c in range(NCORES)
        ],
        axis=0,
    )
    return out, res


def bench(input_batch, table, reps=20, nc=None, chain=1):
    """Time repeated on-device executions (inputs device-resident, no
    donation, no host transfers in the timed region). Returns per-exec
    seconds (min over reps) which includes the axon dispatch round trip."""
    import time

    import jax
    from jax.sharding import Mesh, NamedSharding, PartitionSpec
    from jax.experimental.shard_map import shard_map

    from concourse import bass2jax
    from concourse.bass2jax import (
        _bass_exec_p,
        install_neuronx_cc_hook,
        partition_id_tensor,
    )

    if nc is None:
        nc = _get_nc()
    install_neuronx_cc_hook()
    in_maps = _in_maps(input_batch, table)

    partition_name = (
        nc.partition_id_tensor.name if nc.partition_id_tensor else None
    )
    in_names, out_names, out_avals, zero_outs = [], [], [], []
    for alloc in nc.m.functions[0].allocations:
        if not isinstance(alloc, mybir.MemoryLocationSet):
            continue
        name = alloc.memorylocations[0].name
        if alloc.kind == "ExternalInput":
            if name != partition_name:
                in_names.append(name)
        elif alloc.kind == "ExternalOutput":
            out_names.append(name)
            shape = tuple(alloc.tensor_shape)
            dtype = mybir.dt.np(alloc.dtype)
            out_avals.append(jax.core.ShapedArray(shape, dtype))
            zero_outs.append(np.zeros(shape, dtype))
    n_params = len(in_names)
    all_in_names = in_names + out_names
    if partition_name is not None:
        all_in_names = all_in_names + [partition_name]

    def _body(*args):
        ins_only = list(args[:n_params])
        outs = list(args[n_params:])
        pid = [partition_id_tensor()] if partition_name is not None else []
        for _ in range(chain):
            operands = ins_only + outs + pid
            outs = list(
                _bass_exec_p.bind(
                    *operands,
                    out_avals=tuple(out_avals),
                    in_names=tuple(all_in_names),
                    out_names=tuple(out_names),
                    lowering_input_output_aliases=(),
                    sim_require_finite=True,
                    sim_require_nnan=True,
                    nc=nc,
                )
            )
        return tuple(outs)

    devices = jax.devices()[:NCORES]
    mesh = Mesh(np.asarray(devices), ("core",))
    nshard = NamedSharding(mesh, PartitionSpec("core"))
    sharded = jax.jit(
        shard_map(
            _body,
            mesh=mesh,
            in_specs=(PartitionSpec("core"),) * (n_params + len(out_names)),
            out_specs=(PartitionSpec("core"),) * len(out_names),
            check_rep=False,
        ),
        keep_unused=True,
    )
    concat_in = [
        np.concatenate([np.asarray(in_maps[c][nm]) for c in range(NCORES)], axis=0)
        for nm in in_names
    ]
    concat_zeros = [
        np.zeros((NCORES * z.shape[0], *z.shape[1:]), z.dtype) for z in zero_outs
    ]
    dev_args = [jax.device_put(a, nshard) for a in concat_in + concat_zeros]
    jax.block_until_ready(dev_args)
    # warmup (compiles NEFF on first call)
    out = sharded(*dev_args)
    jax.block_until_ready(out)
    times = []
    for _ in range(reps):
        t0 = time.perf_counter()
        out = sharded(*dev_args)
        jax.block_until_ready(out)
        times.append(time.perf_counter() - t0)
    return min(times), times, out


# revision 8
# speedup vs baseline: 1.1914x; 1.1914x over previous
"""Embedding lookup (nn_LookupNetwork) on 8 Trainium2 NeuronCores.

Strategy: data-parallel over the batch; each core handles 512 of the 4096
batch rows (102,400 lookups) with a replicated table in local HBM.

The gather uses the Q7 `dma_gather` SWDGE instruction (one descriptor per
looked-up row, streamed through all 16 SDMA engines) instead of per-column
indirect DMAs. dma_gather indices are int16, so the 100k-row table is
re-laid-out on the host into 4 chunks of 32767 rows + 1 zero row each
(device table [4*32768, 128]); every position is gathered once per chunk,
with out-of-chunk positions (and the -1 sentinel) pointed at the chunk's
zero row. The 4 gathered tiles are summed on the DVE - no masking needed.

dma_gather writes element i to partition i%128, block i//128; the host
pre-permutes the index slots so that the summed SBUF tile is already in
natural row order per partition, making the output store a contiguous
128 x 16KB HWDGE DMA and the host-side unshard a pure reshape.
"""

import sys

sys.path.insert(0, "/opt/trn_rl_repo")

from contextlib import ExitStack

import numpy as np

import concourse.bacc as bacc
import concourse.bass as bass
import concourse.mybir as mybir
import concourse.tile as tile
from concourse.bass_utils import run_bass_kernel_spmd

VOCAB, D = 100000, 128
BATCH, HIST = 4096, 200
NCORES = 8
P = 128

V_CH = 32767            # table rows per chunk (int16 max + zero row)
NCH = 4                 # chunks needed to cover VOCAB
TBL_CH = 32768          # device-table rows per chunk (incl. zero row)
S = 4096                # positions gathered per tile
BLK = S // P            # output rows per partition per tile
NPOS = BATCH * HIST // NCORES   # 102400 positions per core
NT = NPOS // S          # tiles per core

_nc_cache = {}


def build_nc(nt=NT, bufs=2, reps=1):
    """reps>1 repeats the whole workload in-program (for timing: the
    per-exec HW time is (t(reps=R) - t(reps=1)) / (R - 1), which cancels
    the host/axon dispatch overhead)."""
    nc = bacc.Bacc(
        "TRN2", target_bir_lowering=False, debug=False, enable_asserts=False,
        num_swdge_queues=4,
    )
    idx_d = nc.dram_tensor(
        "idx", [P, nt * NCH * (S // 16)], mybir.dt.int16, kind="ExternalInput"
    ).ap()
    tab_d = nc.dram_tensor(
        "table", [NCH * TBL_CH, D], mybir.dt.float32, kind="ExternalInput"
    ).ap()
    out_d = nc.dram_tensor(
        "out", [nt, P, S], mybir.dt.float32, kind="ExternalOutput"
    ).ap()

    cols_t = NCH * (S // 16)        # idx columns per tile (all 4 chunks)
    with tile.TileContext(nc) as tc:
        with ExitStack() as ctx:
            ipool = ctx.enter_context(tc.tile_pool(name="ipool", bufs=3))
            gpools = [
                ctx.enter_context(tc.tile_pool(name=f"g{c}", bufs=bufs))
                for c in range(NCH)
            ]
            for t in [t for _ in range(reps) for t in range(nt)]:
                idx_t = ipool.tile([P, cols_t], mybir.dt.int16)
                nc.scalar.dma_start(
                    idx_t[:], idx_d[:, t * cols_t : (t + 1) * cols_t]
                )
                gs = []
                for c in range(NCH):
                    g = gpools[c].tile([P, S], mybir.dt.float32)
                    g3 = g[:].rearrange("p (b d) -> p b d", d=D)
                    nc.gpsimd.dma_gather(
                        g3,
                        tab_d[c * TBL_CH : (c + 1) * TBL_CH, :],
                        idx_t[:, c * (S // 16) : (c + 1) * (S // 16)],
                        S,
                        S,
                        D,
                        # >64 descriptors per lane per instruction cannot be
                        # coalesced into one SDMA packet
                        single_packet=False,
                        # one SWDGE queue is Q7-descgen-bound (~27us per
                        # 4096-idx gather); spread chunks over all 4 queues
                        queue_num=c,
                    )
                    gs.append(g)
                nc.vector.tensor_tensor(
                    out=gs[0][:], in0=gs[0][:], in1=gs[1][:],
                    op=mybir.AluOpType.add,
                )
                nc.vector.tensor_tensor(
                    out=gs[2][:], in0=gs[2][:], in1=gs[3][:],
                    op=mybir.AluOpType.add,
                )
                nc.vector.tensor_tensor(
                    out=gs[0][:], in0=gs[0][:], in1=gs[2][:],
                    op=mybir.AluOpType.add,
                )
                nc.sync.dma_start(out_d[t], gs[0][:])
    nc.compile()
    return nc


def _get_nc():
    if "nc" not in _nc_cache:
        _nc_cache["nc"] = build_nc()
    return _nc_cache["nc"]


def _prep_table(table):
    t4 = np.zeros((NCH * TBL_CH, D), np.float32)
    for c in range(NCH):
        lo = c * V_CH
        hi = min(lo + V_CH, VOCAB)
        t4[c * TBL_CH : c * TBL_CH + (hi - lo)] = table[lo:hi]
    return t4


def _in_maps(input_batch, table, nt=NT):
    # slot i (within a tile) holds position p*BLK+b with p=i%128, b=i//128,
    # so dma_gather's i%128-partition placement lands rows in natural order
    # per partition. wrap-16: slot i sits at idx column i//16, partition i%16.
    v = np.asarray(input_batch).astype(np.int64).reshape(NCORES, NT, P, BLK)
    v = v[:, :nt]
    slots = v.transpose(0, 1, 3, 2).reshape(NCORES, nt, S)
    w = slots.reshape(NCORES, nt, S // 16, 16).transpose(0, 1, 3, 2)
    chunks = [
        np.where((w >= 0) & (w // V_CH == c), w - c * V_CH, V_CH).astype(np.int16)
        for c in range(NCH)
    ]
    idx = np.stack(chunks, axis=2)              # [core, nt, NCH, 16, 256]
    # device layout: partition q (= q%16 replica), column t*NCH*256 + c*256 + s
    idx = idx.transpose(0, 3, 1, 2, 4)          # [core, 16, nt, NCH, 256]
    idx = idx.reshape(NCORES, 16, nt * NCH * (S // 16))
    idx = np.ascontiguousarray(np.tile(idx, (1, 8, 1)))   # replicate to 128
    t4 = _prep_table(np.asarray(table, dtype=np.float32))
    return [{"idx": idx[c], "table": t4} for c in range(NCORES)]


def kernel(input_batch, table):
    nc = _get_nc()
    in_maps = _in_maps(input_batch, table)
    res = run_bass_kernel_spmd(nc, in_maps, list(range(NCORES)))
    return np.concatenate(
        [
            res.results[c]["out"].reshape(NPOS // HIST, HIST, D)
            for c in range(NCORES)
        ],
        axis=0,
    )


def run_traced(input_batch, table, trace_cores=None, tmpdir=None):
    """Run once with NTFF profiling; returns (output, BassKernelResults)."""
    nc = _get_nc()
    in_maps = _in_maps(input_batch, table)
    res = run_bass_kernel_spmd(
        nc, in_maps, list(range(NCORES)), trace=True,
        trace_cores=trace_cores, tmpdir=tmpdir,
    )
    out = np.concatenate(
        [
            res.results[c]["out"].reshape(NPOS // HIST, HIST, D)
            for c in range(NCORES)
        ],
        axis=0,
    )
    return out, res


def bench(input_batch, table, reps=20, nc=None, chain=1):
    """Time repeated on-device executions (inputs device-resident, no
    donation, no host transfers in the timed region). Returns per-exec
    seconds (min over reps) which includes the axon dispatch round trip."""
    import time

    import jax
    from jax.sharding import Mesh, NamedSharding, PartitionSpec
    from jax.experimental.shard_map import shard_map

    from concourse import bass2jax
    from concourse.bass2jax import (
        _bass_exec_p,
        install_neuronx_cc_hook,
        partition_id_tensor,
    )

    if nc is None:
        nc = _get_nc()
    install_neuronx_cc_hook()
    in_maps = _in_maps(input_batch, table)

    partition_name = (
        nc.partition_id_tensor.name if nc.partition_id_tensor else None
    )
    in_names, out_names, out_avals, zero_outs = [], [], [], []
    for alloc in nc.m.functions[0].allocations:
        if not isinstance(alloc, mybir.MemoryLocationSet):
            continue
        name = alloc.memorylocations[0].name
        if alloc.kind == "ExternalInput":
            if name != partition_name:
                in_names.append(name)
        elif alloc.kind == "ExternalOutput":
            out_names.append(name)
            shape = tuple(alloc.tensor_shape)
            dtype = mybir.dt.np(alloc.dtype)
            out_avals.append(jax.core.ShapedArray(shape, dtype))
            zero_outs.append(np.zeros(shape, dtype))
    n_params = len(in_names)
    all_in_names = in_names + out_names
    if partition_name is not None:
        all_in_names = all_in_names + [partition_name]

    def _body(*args):
        ins_only = list(args[:n_params])
        outs = list(args[n_params:])
        pid = [partition_id_tensor()] if partition_name is not None else []
        for _ in range(chain):
            operands = ins_only + outs + pid
            outs = list(
                _bass_exec_p.bind(
                    *operands,
                    out_avals=tuple(out_avals),
                    in_names=tuple(all_in_names),
                    out_names=tuple(out_names),
                    lowering_input_output_aliases=(),
                    sim_require_finite=True,
                    sim_require_nnan=True,
                    nc=nc,
                )
            )
        return tuple(outs)

    devices = jax.devices()[:NCORES]
    mesh = Mesh(np.asarray(devices), ("core",))
    nshard = NamedSharding(mesh, PartitionSpec("core"))
    sharded = jax.jit(
        shard_map(
            _body,
            mesh=mesh,
            in_specs=(PartitionSpec("core"),) * (n_params + len(out_names)),
            out_specs=(PartitionSpec("core"),) * len(out_names),
            check_rep=False,
        ),
        keep_unused=True,
    )
    concat_in = [
        np.concatenate([np.asarray(in_maps[c][nm]) for c in range(NCORES)], axis=0)
        for nm in in_names
    ]
    concat_zeros = [
        np.zeros((NCORES * z.shape[0], *z.shape[1:]), z.dtype) for z in zero_outs
    ]
    dev_args = [jax.device_put(a, nshard) for a in concat_in + concat_zeros]
    jax.block_until_ready(dev_args)
    # warmup (compiles NEFF on first call)
    out = sharded(*dev_args)
    jax.block_until_ready(out)
    times = []
    for _ in range(reps):
        t0 = time.perf_counter()
        out = sharded(*dev_args)
        jax.block_until_ready(out)
        times.append(time.perf_counter() - t0)
    return min(times), times, out
